# revision 23
# baseline (speedup 1.0000x reference)
"""Bass/Trainium2 kernel for nn_Attention_73641509257513.

Reference op: y = LayerNorm(x; g_ln) -> per-head (H=8, head_dim=E=512)
causal attention -> output projection Wo. B=4, S=2048, E=512.

Sharding: Megatron tensor-parallel over heads - 1 head per NeuronCore
(8 cores). Each core:
  - LayerNorm of all tokens (replicated; g_ln folded into Wq/Wk/Wv on host)
  - Q_T/K_T projections + V (contraction dims on partitions everywhere,
    xn transposed via DMA-xbar through a DRAM bounce)
  - causal attention for its head, scores computed transposed S_T[k,q]
    (softmax denominator via ones-matmul on PE; causal mask via
    affine_select; no max-subtraction - scores are provably small)
  - partial output Y_h = ctx_T.T @ Wo_h, normalized by 1/denominator late
  - per-batch ReduceScatter(add) across the 8 cores
Host reassembles the 8 row-shards.
"""

import math

import numpy as np
import ml_dtypes

import concourse.bacc as bacc
import concourse.mybir as mybir
import concourse.tile as tile
from concourse.bass_utils import run_bass_kernel_spmd

B, S, E, H = 4, 2048, 512, 8
P = 128
EC = E // P            # 4 e-chunks
TT = S // P            # 16 token tiles per batch
QT = S // 512          # 4 q-tiles (512 wide) per batch
NCORES = 8
LN_EPS = 1e-5
SCALE = 1.0 / math.sqrt(E)
SHARD = S // NCORES    # 256 rows per batch per core

BF16 = mybir.dt.bfloat16
F32 = mybir.dt.float32

# build options (tuned via TimelineSim sweep)
OPTS = {
    "ln_mode": "after",      # after | before | inter
    "proj_copy": "dve",      # dve | act
    "ctx_copy": "act",       # dve | act
    "yscale": "act",         # dve | act
    "diag_first": True,
    "expt_bufs": 18,
    "last_rs_halves": 4,
    "rs_mode": "one",  # batch_last4 | batch | two | one
    "ln_group": 4,
    # timing-bisect switches (produce WRONG results; timing only)
    "no_cc": False,
    "no_mask": False,
    "no_tr": False,
    "no_den": False,
    "no_ln": False,
}


def build_nc():
    nc = bacc.Bacc("TRN2", target_bir_lowering=False, debug=False,
                   num_devices=NCORES)
    x_in = nc.dram_tensor("x", [B * S, E], F32, kind="ExternalInput").ap()
    wq_in = nc.dram_tensor("wq", [E, E], BF16, kind="ExternalInput").ap()
    wk_in = nc.dram_tensor("wk", [E, E], BF16, kind="ExternalInput").ap()
    wv_in = nc.dram_tensor("wv", [E, E], BF16, kind="ExternalInput").ap()
    wo_in = nc.dram_tensor("wo", [E, E], BF16, kind="ExternalInput").ap()
    y_out = nc.dram_tensor("y", [B * SHARD, E], F32, kind="ExternalOutput").ap()

    with tile.TileContext(nc) as tc:
        with (tc.tile_pool(name="consts", bufs=1) as consts,
              tc.tile_pool(name="ln", bufs=3) as ln,
              tc.tile_pool(name="lnst", bufs=4) as lnst,
              tc.tile_pool(name="xnt", bufs=2) as xnt,
              tc.tile_pool(name="qkv", bufs=2) as qkv,
              tc.tile_pool(name="expt", bufs=OPTS["expt_bufs"]) as expt,
              tc.tile_pool(name="ctxp", bufs=2) as ctxp,
              tc.tile_pool(name="yout", bufs=4) as yout,
              tc.tile_pool(name="denp", bufs=4) as denp,
              tc.tile_pool(name="ps_mm", bufs=2, space="PSUM") as ps_mm,
              tc.tile_pool(name="ps_sc", bufs=2, space="PSUM") as ps_sc,
              tc.tile_pool(name="ps_ctx", bufs=2, space="PSUM") as ps_ctx,
              tc.tile_pool(name="ps_den", bufs=2, space="PSUM") as ps_den,
              tc.tile_pool(name="dram", bufs=2, space="DRAM") as dram):

            # weights: [e_in, e_out] -> sbuf [128, EC, 512] (chunk c = rows 128c..)
            w_sb = {}
            for name, w_ap in (("wq", wq_in), ("wk", wk_in),
                               ("wv", wv_in), ("wo", wo_in)):
                t = consts.tile([P, EC, E], BF16, name=f"{name}_sb")
                nc.sync.dma_start(out=t, in_=w_ap.rearrange("(c p) n -> p c n", p=P))
                w_sb[name] = t
            ones_sb = consts.tile([P, 1], BF16)
            nc.vector.memset(ones_sb, 1.0)
            eps_sb = consts.tile([P, 1], F32)
            nc.vector.memset(eps_sb, LN_EPS)

            def alloc_xn_dram(b):
                return dram.tile([S, E], BF16, tag="xnd", name=f"xnd{b}")

            def emit_ln_tiles(b, xn_dram, tiles):
                """LayerNorm of given token tiles of batch b -> xn_dram.

                Stats per tile on DVE; sqrt batched across OPTS["ln_group"]
                tiles into one ACT op (fewer act-table switch points)."""
                row0 = b * S
                tiles = list(tiles)
                g = OPTS["ln_group"]
                for i0 in range(0, len(tiles), g):
                    grp = tiles[i0: i0 + g]
                    ng = len(grp)
                    x_tiles = {}
                    mv_all = lnst.tile([P, g, 2], F32, tag="mv",
                                       name=f"mv{b}_{grp[0]}")
                    for gi, t in enumerate(grp):
                        x_tile = ln.tile([P, E], F32, tag="x", name=f"x{b}_{t}",
                                         bufs=OPTS["ln_group"] + 2)
                        nc.sync.dma_start(
                            out=x_tile,
                            in_=x_in[row0 + P * t: row0 + P * (t + 1), :])
                        x_tiles[t] = x_tile
                        stats = lnst.tile([P, nc.vector.BN_STATS_DIM], F32,
                                          tag="st", name=f"st{b}_{t}")
                        nc.vector.bn_stats(out=stats, in_=x_tile)
                        nc.vector.bn_aggr(out=mv_all[:, gi, :], in_=stats)
                    std_all = lnst.tile([P, g], F32, tag="sd",
                                        name=f"sd{b}_{grp[0]}")
                    nc.scalar.activation(std_all[:, :ng], mv_all[:, :ng, 1],
                                         mybir.ActivationFunctionType.Sqrt,
                                         bias=eps_sb)
                    rs_all = lnst.tile([P, g], F32, tag="rs",
                                       name=f"rs{b}_{grp[0]}")
                    nc.vector.reciprocal(rs_all[:, :ng], std_all[:, :ng])
                    for gi, t in enumerate(grp):
                        xn_tile = ln.tile([P, E], BF16, tag="xn",
                                          name=f"xn{b}_{t}")
                        nc.vector.tensor_scalar(
                            xn_tile, x_tiles[t], mv_all[:, gi, 0:1],
                            rs_all[:, gi: gi + 1],
                            mybir.AluOpType.subtract,
                            mybir.AluOpType.mult)
                        nc.sync.dma_start(out=xn_dram[P * t: P * (t + 1), :],
                                          in_=xn_tile)

            def emit_proj(b, xn_dram):
                """Transpose loads + Q_T/K_T/V projections for batch b."""
                xn_t = xnt.tile([P, EC, S], BF16, tag="xnt", name=f"xnt{b}")
                qt_sb = qkv.tile([P, EC, S], BF16, tag="qt", name=f"qt{b}")
                kt_sb = qkv.tile([P, EC, S], BF16, tag="kt", name=f"kt{b}")
                v_sb = qkv.tile([P, TT, E], BF16, tag="v", name=f"v{b}")
                for tg in range(S // 512):
                    for c in range(EC):
                        if OPTS["no_tr"]:
                            nc.sync.dma_start(
                                out=xn_t[:, c, 512 * tg: 512 * (tg + 1)],
                                in_=xn_dram[0: P, 0: 512])
                        else:
                            nc.sync.dma_start_transpose(
                                out=xn_t[:, c, 512 * tg: 512 * (tg + 1)],
                                in_=xn_dram[512 * tg: 512 * (tg + 1),
                                            P * c: P * (c + 1)])
                    for dst, wname in ((qt_sb, "wq"), (kt_sb, "wk")):
                        w = w_sb[wname]
                        for eo in range(EC):
                            ps = ps_mm.tile([P, 512], F32, tag="mm",
                                            name=f"pj{b}_{tg}_{wname}{eo}")
                            for ei in range(EC):
                                nc.tensor.matmul(
                                    ps,
                                    w[:, ei, P * eo: P * (eo + 1)],
                                    xn_t[:, ei, 512 * tg: 512 * (tg + 1)],
                                    start=(ei == 0), stop=(ei == EC - 1))
                            (nc.scalar.copy if OPTS["proj_copy"] == "act"
                             else nc.vector.tensor_copy)(
                                dst[:, eo, 512 * tg: 512 * (tg + 1)], ps)
                    for t in range(4 * tg, 4 * (tg + 1)):
                        ps = ps_mm.tile([P, 512], F32, tag="mm",
                                        name=f"pv{b}_{t}")
                        for ei in range(EC):
                            nc.tensor.matmul(ps,
                                             xn_t[:, ei, P * t: P * (t + 1)],
                                             w_sb["wv"][:, ei, :],
                                             start=(ei == 0), stop=(ei == EC - 1))
                        (nc.scalar.copy if OPTS["proj_copy"] == "act"
                         else nc.vector.tensor_copy)(v_sb[:, t, :], ps)
                return qt_sb, kt_sb, v_sb

            def emit_attn(b, qt_sb, kt_sb, v_sb, y_all, ln_hook=None):
                """Causal attention + output projection into y_all."""
                for qt in range(QT):
                    if ln_hook is not None:
                        ln_hook(qt)
                    q0 = 512 * qt
                    nkc = (q0 + 512) // P
                    # diagonal (masked) chunks first so their affine_selects
                    # complete while the dense chunks' scores still stream
                    if OPTS["diag_first"]:
                        kcs = list(range(4 * qt, nkc)) + list(range(4 * qt))
                    else:
                        kcs = list(range(nkc))
                    ex_by_kc = {}
                    for kc in kcs:
                        ps_s = ps_sc.tile([P, 512], F32, tag="sc",
                                          name=f"sc{b}_{qt}_{kc}")
                        for ei in range(EC):
                            nc.tensor.matmul(
                                ps_s,
                                kt_sb[:, ei, P * kc: P * (kc + 1)],
                                qt_sb[:, ei, q0: q0 + 512],
                                start=(ei == 0), stop=(ei == EC - 1))
                        ex = expt.tile([P, 512], BF16, tag="ex",
                                       name=f"ex{b}_{qt}_{kc}")
                        nc.scalar.activation(ex, ps_s,
                                             mybir.ActivationFunctionType.Exp,
                                             scale=SCALE)
                        j = kc - 4 * qt
                        if j >= 0 and not OPTS["no_mask"]:
                            nc.gpsimd.affine_select(
                                out=ex, in_=ex, pattern=[[1, 512]],
                                compare_op=mybir.AluOpType.is_ge, fill=0.0,
                                base=-(P * j), channel_multiplier=-1)
                        ex_by_kc[kc] = ex

                    ps_d = ps_den.tile([1, 512], F32, tag="den",
                                       name=f"dn{b}_{qt}")
                    for i, kc in enumerate(kcs):
                        nc.tensor.matmul(ps_d, ones_sb, ex_by_kc[kc],
                                         start=(i == 0), stop=(i == nkc - 1))
                    den_sb = denp.tile([1, 512], F32, tag="dsb",
                                       name=f"ds{b}_{qt}")
                    nc.vector.tensor_copy(den_sb, ps_d)
                    recip = denp.tile([P, 4], F32, tag="rcp",
                                      name=f"rc{b}_{qt}")
                    if OPTS["no_den"]:
                        nc.vector.memset(recip, 1.0)
                    else:
                        den_dr = dram.tile([512], F32, tag="dend", bufs=4,
                                           name=f"dd{b}_{qt}")
                        nc.sync.dma_start(out=den_dr, in_=den_sb)
                        den_pj = denp.tile([P, 4], F32, tag="dpj",
                                           name=f"dp{b}_{qt}")
                        nc.sync.dma_start(out=den_pj,
                                          in_=den_dr.rearrange("(j p) -> p j", p=P))
                        nc.vector.reciprocal(recip, den_pj)

                    ctx_sb = ctxp.tile([P, EC, 512], BF16, tag="ctx",
                                       name=f"cx{b}_{qt}")
                    for ec in range(EC):
                        ps_c = ps_ctx.tile([P, 512], F32, tag="ctx",
                                           name=f"pc{b}_{qt}_{ec}")
                        for i, kc in enumerate(kcs):
                            nc.tensor.matmul(
                                ps_c,
                                v_sb[:, kc, P * ec: P * (ec + 1)],
                                ex_by_kc[kc],
                                start=(i == 0), stop=(i == nkc - 1))
                        (nc.scalar.copy if OPTS["ctx_copy"] == "act"
                         else nc.vector.tensor_copy)(ctx_sb[:, ec, :], ps_c)

                    for jj in range(4):
                        ps_y = ps_mm.tile([P, 512], F32, tag="mm",
                                          name=f"py{b}_{qt}_{jj}")
                        for ec in range(EC):
                            nc.tensor.matmul(
                                ps_y,
                                ctx_sb[:, ec, P * jj: P * (jj + 1)],
                                w_sb["wo"][:, ec, :],
                                start=(ec == 0), stop=(ec == EC - 1))
                        y_tile = yout.tile([P, 512], F32, tag="yt",
                                           name=f"yt{b}_{qt}_{jj}")
                        if OPTS["yscale"] == "act":
                            nc.scalar.mul(y_tile, ps_y, recip[:, jj: jj + 1])
                        else:
                            nc.vector.tensor_scalar_mul(y_tile, ps_y,
                                                        recip[:, jj: jj + 1])
                        r0 = b * S + q0 + P * jj
                        nc.sync.dma_start(out=y_all[r0: r0 + P, :], in_=y_tile)

            def emit_rs(y_all, r0, nrows, out_r0):
                """ReduceScatter y_all[r0:r0+nrows] -> y_out[out_r0:...]."""
                rs_o = dram.tile([nrows // NCORES, E], F32, tag="rso",
                                 bufs=3, name=f"rs{r0}")
                if OPTS["no_cc"]:
                    nc.sync.dma_start(
                        out=rs_o, in_=y_all[r0: r0 + nrows // NCORES, :])
                else:
                    nc.gpsimd.collective_compute(
                        "ReduceScatter", mybir.AluOpType.add,
                        replica_groups=[list(range(NCORES))],
                        ins=[y_all[r0: r0 + nrows, :].rearrange("s e -> (s e)")],
                        outs=[rs_o.rearrange("s e -> (s e)")])
                nc.sync.dma_start(
                    out=y_out[out_r0: out_r0 + nrows // NCORES, :], in_=rs_o)

            # ReduceScatter piece list: (global_row0, nrows), emitted after
            # the batch whose index is the dict key completes.
            if OPTS["rs_mode"] == "batch_last4":
                pieces = {0: [(0, S)], 1: [(S, S)], 2: [(2 * S, S)],
                          3: [(3 * S + (S // 4) * i, S // 4) for i in range(4)]}
            elif OPTS["rs_mode"] == "batch":
                pieces = {b: [(b * S, S)] for b in range(B)}
            elif OPTS["rs_mode"] == "two":
                pieces = {1: [(0, 2 * S)], 3: [(2 * S, 2 * S)]}
            elif OPTS["rs_mode"] == "one":
                pieces = {3: [(0, B * S)]}
            else:
                raise ValueError(OPTS["rs_mode"])

            # pipeline: LN(b+1) relative to attention(b) per OPTS["ln_mode"]
            y_all = dram.tile([B * S, E], F32, tag="yall", bufs=1)
            xn_cur = alloc_xn_dram(0)
            emit_ln_tiles(0, xn_cur, range(TT))
            out_r0 = 0
            for b in range(B):
                pk = emit_proj(b, xn_cur)
                hook = None
                if b + 1 < B:
                    xn_nxt = alloc_xn_dram(b + 1)
                    if OPTS["ln_mode"] == "before":
                        emit_ln_tiles(b + 1, xn_nxt, range(TT))
                    elif OPTS["ln_mode"] == "inter":
                        def hook(qt, _b=b + 1, _xd=xn_nxt):
                            emit_ln_tiles(_b, _xd, range(4 * qt, 4 * (qt + 1)))
                    xn_cur = xn_nxt
                emit_attn(b, *pk, y_all, ln_hook=hook)
                if b + 1 < B and OPTS["ln_mode"] == "after":
                    emit_ln_tiles(b + 1, xn_cur, range(TT))
                for (pr0, pn) in pieces.get(b, []):
                    emit_rs(y_all, pr0, pn, out_r0)
                    out_r0 += pn // NCORES

    nc.finalize()
    return nc


_NC_CACHE = None


def _get_nc():
    global _NC_CACHE
    if _NC_CACHE is None:
        _NC_CACHE = build_nc()
    return _NC_CACHE


def make_in_maps(inputs):
    """Host-side sharding: slice/cast per-core weights, fold g_ln."""
    x = np.asarray(inputs["x"], dtype=np.float32)
    g_ln = np.asarray(inputs["g_ln"], dtype=np.float32)
    Wq = np.asarray(inputs["Wq"], dtype=np.float32)
    Wk = np.asarray(inputs["Wk"], dtype=np.float32)
    Wv = np.asarray(inputs["Wv"], dtype=np.float32)
    Wo = np.asarray(inputs["Wo"], dtype=np.float32)

    x2 = np.ascontiguousarray(x.reshape(B * S, E))
    g = g_ln[:, None]
    in_maps = []
    for h in range(NCORES):
        sl = slice(E * h, E * (h + 1))
        in_maps.append({
            "x": x2,
            "wq": np.ascontiguousarray(g * Wq[:, sl]).astype(ml_dtypes.bfloat16),
            "wk": np.ascontiguousarray(g * Wk[:, sl]).astype(ml_dtypes.bfloat16),
            "wv": np.ascontiguousarray(g * Wv[:, sl]).astype(ml_dtypes.bfloat16),
            "wo": np.ascontiguousarray(Wo[sl, :]).astype(ml_dtypes.bfloat16),
        })
    return in_maps


def kernel(**inputs) -> np.ndarray:
    in_maps = make_in_maps(inputs)
    nc = _get_nc()
    res = run_bass_kernel_spmd(nc, in_maps, core_ids=list(range(NCORES)))

    if OPTS["rs_mode"] == "batch_last4":
        pieces = [(0, S), (S, S), (2 * S, S)] + \
            [(3 * S + (S // 4) * i, S // 4) for i in range(4)]
    elif OPTS["rs_mode"] == "batch":
        pieces = [(b * S, S) for b in range(B)]
    elif OPTS["rs_mode"] == "two":
        pieces = [(0, 2 * S), (2 * S, 2 * S)]
    elif OPTS["rs_mode"] == "one":
        pieces = [(0, B * S)]
    y2 = np.empty((B * S, E), dtype=np.float32)
    out_r0 = 0
    for (pr0, pn) in pieces:
        sh = pn // NCORES
        for c in range(NCORES):
            y2[pr0 + sh * c: pr0 + sh * (c + 1), :] = \
                res.results[c]["y"][out_r0: out_r0 + sh, :]
        out_r0 += sh
    return y2.reshape(B, S, E)


# revision 25
# speedup vs baseline: 1.1169x; 1.1169x over previous
"""Bass/Trainium2 kernel for nn_Attention_73641509257513.

Reference op: y = LayerNorm(x; g_ln) -> per-head (H=8, head_dim=E=512)
causal attention -> output projection Wo. B=4, S=2048, E=512.

Sharding: Megatron tensor-parallel over heads - 1 head per NeuronCore
(8 cores). Each core:
  - LayerNorm of all tokens (replicated; g_ln folded into Wq/Wk/Wv on host)
  - Q_T/K_T projections + V (contraction dims on partitions everywhere,
    xn transposed via DMA-xbar through a DRAM bounce)
  - causal attention for its head, scores computed transposed S_T[k,q]
    (softmax denominator via ones-matmul on PE; causal mask via
    affine_select; no max-subtraction - scores are provably small)
  - partial output Y_h = ctx_T.T @ Wo_h, normalized by 1/denominator late
  - per-batch ReduceScatter(add) across the 8 cores
Host reassembles the 8 row-shards.
"""

import math

import numpy as np
import ml_dtypes

import concourse.bacc as bacc
import concourse.mybir as mybir
import concourse.tile as tile
from concourse.bass_utils import run_bass_kernel_spmd

B, S, E, H = 4, 2048, 512, 8
P = 128
EC = E // P            # 4 e-chunks
TT = S // P            # 16 token tiles per batch
QT = S // 512          # 4 q-tiles (512 wide) per batch
NCORES = 8
LN_EPS = 1e-5
SCALE = 1.0 / math.sqrt(E)
SHARD = S // NCORES    # 256 rows per batch per core

BF16 = mybir.dt.bfloat16
F32 = mybir.dt.float32

# build options (tuned via TimelineSim sweep)
OPTS = {
    "ln_mode": "after",      # after | before | inter
    "proj_copy": "dve",      # dve | act
    "ctx_copy": "act",       # dve | act
    "yscale": "act",         # dve | act
    "diag_first": True,
    "expt_bufs": 18,
    "last_rs_halves": 4,
    "rs_mode": "one",  # batch_last4 | batch | two | one
    "ln_group": 4,
    "ln0_group": 2,
    "ps_sc_bufs": 3,
    "ps_den_bufs": 1,
    # timing-bisect switches (produce WRONG results; timing only)
    "no_cc": False,
    "no_mask": False,
    "no_tr": False,
    "no_den": False,
    "no_ln": False,
}


def build_nc():
    nc = bacc.Bacc("TRN2", target_bir_lowering=False, debug=False,
                   num_devices=NCORES)
    x_in = nc.dram_tensor("x", [B * S, E], F32, kind="ExternalInput").ap()
    wq_in = nc.dram_tensor("wq", [E, E], BF16, kind="ExternalInput").ap()
    wk_in = nc.dram_tensor("wk", [E, E], BF16, kind="ExternalInput").ap()
    wv_in = nc.dram_tensor("wv", [E, E], BF16, kind="ExternalInput").ap()
    wo_in = nc.dram_tensor("wo", [E, E], BF16, kind="ExternalInput").ap()
    y_out = nc.dram_tensor("y", [B * SHARD, E], F32, kind="ExternalOutput").ap()

    with tile.TileContext(nc) as tc:
        with (tc.tile_pool(name="consts", bufs=1) as consts,
              tc.tile_pool(name="ln", bufs=3) as ln,
              tc.tile_pool(name="lnst", bufs=4) as lnst,
              tc.tile_pool(name="xnt", bufs=2) as xnt,
              tc.tile_pool(name="qkv", bufs=2) as qkv,
              tc.tile_pool(name="expt", bufs=OPTS["expt_bufs"]) as expt,
              tc.tile_pool(name="ctxp", bufs=2) as ctxp,
              tc.tile_pool(name="yout", bufs=4) as yout,
              tc.tile_pool(name="denp", bufs=4) as denp,
              tc.tile_pool(name="ps_mm", bufs=2, space="PSUM") as ps_mm,
              tc.tile_pool(name="ps_sc", bufs=OPTS["ps_sc_bufs"],
                           space="PSUM") as ps_sc,
              tc.tile_pool(name="ps_ctx", bufs=2, space="PSUM") as ps_ctx,
              tc.tile_pool(name="ps_den", bufs=OPTS["ps_den_bufs"],
                           space="PSUM") as ps_den,
              tc.tile_pool(name="dram", bufs=2, space="DRAM") as dram):

            # weights: [e_in, e_out] -> sbuf [128, EC, 512] (chunk c = rows 128c..)
            w_sb = {}
            for name, w_ap in (("wq", wq_in), ("wk", wk_in),
                               ("wv", wv_in), ("wo", wo_in)):
                t = consts.tile([P, EC, E], BF16, name=f"{name}_sb")
                nc.sync.dma_start(out=t, in_=w_ap.rearrange("(c p) n -> p c n", p=P))
                w_sb[name] = t
            ones_sb = consts.tile([P, 1], BF16)
            nc.vector.memset(ones_sb, 1.0)
            eps_sb = consts.tile([P, 1], F32)
            nc.vector.memset(eps_sb, LN_EPS)

            def alloc_xn_dram(b):
                return dram.tile([S, E], BF16, tag="xnd", name=f"xnd{b}")

            def emit_ln_tiles(b, xn_dram, tiles, group=None):
                """LayerNorm of given token tiles of batch b -> xn_dram.

                Stats per tile on DVE; sqrt batched across `group` tiles
                into one ACT op (fewer act-table switch points)."""
                row0 = b * S
                tiles = list(tiles)
                g = group or OPTS["ln_group"]
                for i0 in range(0, len(tiles), g):
                    grp = tiles[i0: i0 + g]
                    ng = len(grp)
                    x_tiles = {}
                    mv_all = lnst.tile([P, g, 2], F32, tag="mv",
                                       name=f"mv{b}_{grp[0]}")
                    for gi, t in enumerate(grp):
                        x_tile = ln.tile([P, E], F32, tag="x", name=f"x{b}_{t}",
                                         bufs=OPTS["ln_group"] + 2)
                        nc.sync.dma_start(
                            out=x_tile,
                            in_=x_in[row0 + P * t: row0 + P * (t + 1), :])
                        x_tiles[t] = x_tile
                        stats = lnst.tile([P, nc.vector.BN_STATS_DIM], F32,
                                          tag="st", name=f"st{b}_{t}")
                        nc.vector.bn_stats(out=stats, in_=x_tile)
                        nc.vector.bn_aggr(out=mv_all[:, gi, :], in_=stats)
                    std_all = lnst.tile([P, g], F32, tag="sd",
                                        name=f"sd{b}_{grp[0]}")
                    nc.scalar.activation(std_all[:, :ng], mv_all[:, :ng, 1],
                                         mybir.ActivationFunctionType.Sqrt,
                                         bias=eps_sb)
                    rs_all = lnst.tile([P, g], F32, tag="rs",
                                       name=f"rs{b}_{grp[0]}")
                    nc.vector.reciprocal(rs_all[:, :ng], std_all[:, :ng])
                    for gi, t in enumerate(grp):
                        xn_tile = ln.tile([P, E], BF16, tag="xn",
                                          name=f"xn{b}_{t}")
                        nc.vector.tensor_scalar(
                            xn_tile, x_tiles[t], mv_all[:, gi, 0:1],
                            rs_all[:, gi: gi + 1],
                            mybir.AluOpType.subtract,
                            mybir.AluOpType.mult)
                        nc.sync.dma_start(out=xn_dram[P * t: P * (t + 1), :],
                                          in_=xn_tile)

            def emit_proj(b, xn_dram):
                """Transpose loads + Q_T/K_T/V projections for batch b."""
                xn_t = xnt.tile([P, EC, S], BF16, tag="xnt", name=f"xnt{b}")
                qt_sb = qkv.tile([P, EC, S], BF16, tag="qt", name=f"qt{b}")
                kt_sb = qkv.tile([P, EC, S], BF16, tag="kt", name=f"kt{b}")
                v_sb = qkv.tile([P, TT, E], BF16, tag="v", name=f"v{b}")
                for tg in range(S // 512):
                    for c in range(EC):
                        if OPTS["no_tr"]:
                            nc.sync.dma_start(
                                out=xn_t[:, c, 512 * tg: 512 * (tg + 1)],
                                in_=xn_dram[0: P, 0: 512])
                        else:
                            nc.sync.dma_start_transpose(
                                out=xn_t[:, c, 512 * tg: 512 * (tg + 1)],
                                in_=xn_dram[512 * tg: 512 * (tg + 1),
                                            P * c: P * (c + 1)])
                    for dst, wname in ((qt_sb, "wq"), (kt_sb, "wk")):
                        w = w_sb[wname]
                        for eo in range(EC):
                            ps = ps_mm.tile([P, 512], F32, tag="mm",
                                            name=f"pj{b}_{tg}_{wname}{eo}")
                            for ei in range(EC):
                                nc.tensor.matmul(
                                    ps,
                                    w[:, ei, P * eo: P * (eo + 1)],
                                    xn_t[:, ei, 512 * tg: 512 * (tg + 1)],
                                    start=(ei == 0), stop=(ei == EC - 1))
                            (nc.scalar.copy if OPTS["proj_copy"] == "act"
                             else nc.vector.tensor_copy)(
                                dst[:, eo, 512 * tg: 512 * (tg + 1)], ps)
                    for t in range(4 * tg, 4 * (tg + 1)):
                        ps = ps_mm.tile([P, 512], F32, tag="mm",
                                        name=f"pv{b}_{t}")
                        for ei in range(EC):
                            nc.tensor.matmul(ps,
                                             xn_t[:, ei, P * t: P * (t + 1)],
                                             w_sb["wv"][:, ei, :],
                                             start=(ei == 0), stop=(ei == EC - 1))
                        (nc.scalar.copy if OPTS["proj_copy"] == "act"
                         else nc.vector.tensor_copy)(v_sb[:, t, :], ps)
                return qt_sb, kt_sb, v_sb

            def emit_attn(b, qt_sb, kt_sb, v_sb, y_all, ln_hook=None):
                """Causal attention + output projection into y_all."""
                for qt in range(QT):
                    if ln_hook is not None:
                        ln_hook(qt)
                    q0 = 512 * qt
                    nkc = (q0 + 512) // P
                    # diagonal (masked) chunks first so their affine_selects
                    # complete while the dense chunks' scores still stream
                    if OPTS["diag_first"]:
                        kcs = list(range(4 * qt, nkc)) + list(range(4 * qt))
                    else:
                        kcs = list(range(nkc))
                    ex_by_kc = {}
                    off_by_kc = {}
                    for kc in kcs:
                        # diagonal chunk j covers keys q0+128j..: queries below
                        # 128j are fully masked, so shrink N to the valid range
                        j = kc - 4 * qt
                        off = P * j if j > 0 else 0
                        w = 512 - off
                        off_by_kc[kc] = off
                        ps_s = ps_sc.tile([P, 512], F32, tag="sc",
                                          name=f"sc{b}_{qt}_{kc}")
                        for ei in range(EC):
                            nc.tensor.matmul(
                                ps_s[:, off:],
                                kt_sb[:, ei, P * kc: P * (kc + 1)],
                                qt_sb[:, ei, q0 + off: q0 + 512],
                                start=(ei == 0), stop=(ei == EC - 1))
                        ex = expt.tile([P, 512], BF16, tag="ex",
                                       name=f"ex{b}_{qt}_{kc}")
                        nc.scalar.activation(ex[:, off:], ps_s[:, off:],
                                             mybir.ActivationFunctionType.Exp,
                                             scale=SCALE)
                        if j >= 0 and not OPTS["no_mask"]:
                            nc.gpsimd.affine_select(
                                out=ex[:, off:], in_=ex[:, off:],
                                pattern=[[1, w]],
                                compare_op=mybir.AluOpType.is_ge, fill=0.0,
                                base=0, channel_multiplier=-1)
                        ex_by_kc[kc] = ex

                    ps_d = ps_den.tile([1, 512], F32, tag="den",
                                       name=f"dn{b}_{qt}")
                    for i, kc in enumerate(kcs):
                        o = off_by_kc[kc]
                        nc.tensor.matmul(ps_d[:, o:], ones_sb,
                                         ex_by_kc[kc][:, o:],
                                         start=(i == 0), stop=(i == nkc - 1))
                    den_sb = denp.tile([1, 512], F32, tag="dsb",
                                       name=f"ds{b}_{qt}")
                    nc.vector.tensor_copy(den_sb, ps_d)
                    recip = denp.tile([P, 4], F32, tag="rcp",
                                      name=f"rc{b}_{qt}")
                    if OPTS["no_den"]:
                        nc.vector.memset(recip, 1.0)
                    else:
                        den_dr = dram.tile([512], F32, tag="dend", bufs=4,
                                           name=f"dd{b}_{qt}")
                        nc.sync.dma_start(out=den_dr, in_=den_sb)
                        den_pj = denp.tile([P, 4], F32, tag="dpj",
                                           name=f"dp{b}_{qt}")
                        nc.sync.dma_start(out=den_pj,
                                          in_=den_dr.rearrange("(j p) -> p j", p=P))
                        nc.vector.reciprocal(recip, den_pj)

                    ctx_sb = ctxp.tile([P, EC, 512], BF16, tag="ctx",
                                       name=f"cx{b}_{qt}")
                    for ec in range(EC):
                        ps_c = ps_ctx.tile([P, 512], F32, tag="ctx",
                                           name=f"pc{b}_{qt}_{ec}")
                        for i, kc in enumerate(kcs):
                            o = off_by_kc[kc]
                            nc.tensor.matmul(
                                ps_c[:, o:],
                                v_sb[:, kc, P * ec: P * (ec + 1)],
                                ex_by_kc[kc][:, o:],
                                start=(i == 0), stop=(i == nkc - 1))
                        (nc.scalar.copy if OPTS["ctx_copy"] == "act"
                         else nc.vector.tensor_copy)(ctx_sb[:, ec, :], ps_c)

                    for jj in range(4):
                        ps_y = ps_mm.tile([P, 512], F32, tag="mm",
                                          name=f"py{b}_{qt}_{jj}")
                        for ec in range(EC):
                            nc.tensor.matmul(
                                ps_y,
                                ctx_sb[:, ec, P * jj: P * (jj + 1)],
                                w_sb["wo"][:, ec, :],
                                start=(ec == 0), stop=(ec == EC - 1))
                        y_tile = yout.tile([P, 512], F32, tag="yt",
                                           name=f"yt{b}_{qt}_{jj}")
                        if OPTS["yscale"] == "act":
                            nc.scalar.mul(y_tile, ps_y, recip[:, jj: jj + 1])
                        else:
                            nc.vector.tensor_scalar_mul(y_tile, ps_y,
                                                        recip[:, jj: jj + 1])
                        r0 = b * S + q0 + P * jj
                        nc.sync.dma_start(out=y_all[r0: r0 + P, :], in_=y_tile)

            def emit_rs(y_all, r0, nrows, out_r0):
                """ReduceScatter y_all[r0:r0+nrows] -> y_out[out_r0:...]."""
                rs_o = dram.tile([nrows // NCORES, E], F32, tag="rso",
                                 bufs=3, name=f"rs{r0}")
                if OPTS["no_cc"]:
                    nc.sync.dma_start(
                        out=rs_o, in_=y_all[r0: r0 + nrows // NCORES, :])
                else:
                    nc.gpsimd.collective_compute(
                        "ReduceScatter", mybir.AluOpType.add,
                        replica_groups=[list(range(NCORES))],
                        ins=[y_all[r0: r0 + nrows, :].rearrange("s e -> (s e)")],
                        outs=[rs_o.rearrange("s e -> (s e)")])
                nc.sync.dma_start(
                    out=y_out[out_r0: out_r0 + nrows // NCORES, :], in_=rs_o)

            # ReduceScatter piece list: (global_row0, nrows), emitted after
            # the batch whose index is the dict key completes.
            if OPTS["rs_mode"] == "batch_last4":
                pieces = {0: [(0, S)], 1: [(S, S)], 2: [(2 * S, S)],
                          3: [(3 * S + (S // 4) * i, S // 4) for i in range(4)]}
            elif OPTS["rs_mode"] == "batch":
                pieces = {b: [(b * S, S)] for b in range(B)}
            elif OPTS["rs_mode"] == "two":
                pieces = {1: [(0, 2 * S)], 3: [(2 * S, 2 * S)]}
            elif OPTS["rs_mode"] == "one":
                pieces = {3: [(0, B * S)]}
            else:
                raise ValueError(OPTS["rs_mode"])

            # pipeline: LN(b+1) relative to attention(b) per OPTS["ln_mode"]
            y_all = dram.tile([B * S, E], F32, tag="yall", bufs=1)
            xn_cur = alloc_xn_dram(0)
            # batch 0 is on the critical path: small LN groups up front so the
            # first transposes/projections start as early as possible
            g0 = OPTS["ln0_group"]
            emit_ln_tiles(0, xn_cur, range(0, 4), group=g0)
            emit_ln_tiles(0, xn_cur, range(4, TT))
            out_r0 = 0
            for b in range(B):
                pk = emit_proj(b, xn_cur)
                hook = None
                if b + 1 < B:
                    xn_nxt = alloc_xn_dram(b + 1)
                    if OPTS["ln_mode"] == "before":
                        emit_ln_tiles(b + 1, xn_nxt, range(TT))
                    elif OPTS["ln_mode"] == "inter":
                        def hook(qt, _b=b + 1, _xd=xn_nxt):
                            emit_ln_tiles(_b, _xd, range(4 * qt, 4 * (qt + 1)))
                    xn_cur = xn_nxt
                emit_attn(b, *pk, y_all, ln_hook=hook)
                if b + 1 < B and OPTS["ln_mode"] == "after":
                    emit_ln_tiles(b + 1, xn_cur, range(TT))
                for (pr0, pn) in pieces.get(b, []):
                    emit_rs(y_all, pr0, pn, out_r0)
                    out_r0 += pn // NCORES

    nc.finalize()
    return nc


_NC_CACHE = None


def _get_nc():
    global _NC_CACHE
    if _NC_CACHE is None:
        _NC_CACHE = build_nc()
    return _NC_CACHE


def make_in_maps(inputs):
    """Host-side sharding: slice/cast per-core weights, fold g_ln."""
    x = np.asarray(inputs["x"], dtype=np.float32)
    g_ln = np.asarray(inputs["g_ln"], dtype=np.float32)
    Wq = np.asarray(inputs["Wq"], dtype=np.float32)
    Wk = np.asarray(inputs["Wk"], dtype=np.float32)
    Wv = np.asarray(inputs["Wv"], dtype=np.float32)
    Wo = np.asarray(inputs["Wo"], dtype=np.float32)

    x2 = np.ascontiguousarray(x.reshape(B * S, E))
    g = g_ln[:, None]
    in_maps = []
    for h in range(NCORES):
        sl = slice(E * h, E * (h + 1))
        in_maps.append({
            "x": x2,
            "wq": np.ascontiguousarray(g * Wq[:, sl]).astype(ml_dtypes.bfloat16),
            "wk": np.ascontiguousarray(g * Wk[:, sl]).astype(ml_dtypes.bfloat16),
            "wv": np.ascontiguousarray(g * Wv[:, sl]).astype(ml_dtypes.bfloat16),
            "wo": np.ascontiguousarray(Wo[sl, :]).astype(ml_dtypes.bfloat16),
        })
    return in_maps


def kernel(**inputs) -> np.ndarray:
    in_maps = make_in_maps(inputs)
    nc = _get_nc()
    res = run_bass_kernel_spmd(nc, in_maps, core_ids=list(range(NCORES)))

    if OPTS["rs_mode"] == "batch_last4":
        pieces = [(0, S), (S, S), (2 * S, S)] + \
            [(3 * S + (S // 4) * i, S // 4) for i in range(4)]
    elif OPTS["rs_mode"] == "batch":
        pieces = [(b * S, S) for b in range(B)]
    elif OPTS["rs_mode"] == "two":
        pieces = [(0, 2 * S), (2 * S, 2 * S)]
    elif OPTS["rs_mode"] == "one":
        pieces = [(0, B * S)]
    y2 = np.empty((B * S, E), dtype=np.float32)
    out_r0 = 0
    for (pr0, pn) in pieces:
        sh = pn // NCORES
        for c in range(NCORES):
            y2[pr0 + sh * c: pr0 + sh * (c + 1), :] = \
                res.results[c]["y"][out_r0: out_r0 + sh, :]
        out_r0 += sh
    return y2.reshape(B, S, E)
